# revision 30
# baseline (speedup 1.0000x reference)
"""Channelwise symmetric Hausdorff distance loss on 8 Trainium2 NeuronCores.

Math (per (batch, channel) pair; x, y are [N, D] point sets):
    d2[n, m] = |x_n|^2 + |y_m|^2 - 2 x_n.y_m
    h = max( max_n min_m d(n,m), max_m min_n d(n,m) )
    answer   = mean over the B*C pairs of h.

Sharding: B*C = 24 pairs, 3 per NeuronCore (data parallel), host gathers.

v3 design (per pair, per core):
  - host-prepped: xtc/ytc fp8 in DoubleRow chunk layout [4, 128, 2, N]
    (chunk c holds contraction rows [256c, 256c+256), xt pre-scaled by -2),
    y2c fp16 [1, N] = |y_m|^2 - 2048 (centered, single row),
    x2c fp32 [128, NT] = |x_n|^2 - 1024 in per-partition layout.
  - ~28 K=1 warmup matmuls run during the initial DMA fill so the PE HAM
    clock-gate is already at 2.4 GHz when real matmuls start.
  - 8 n-blocks, each:
      PE:      8 accumulating fp8-DR matmuls (-2 x.y) + 2 K=1 fp16
               fold-ins (ones^T @ y2c) packed CONCURRENTLY on array rows
               0 and 32 -> psum = y2c - 2 x.y  (fp32)
      ScalarE: scr2 = cast(psum + x2c[b]) to fp16 (activation Identity
               with per-partition bias) = d2 - 3072
      DVE:     rowacc[:, b] = min over m of scr2   (tensor_reduce)
      DVE/GpSimd (alternating blocks): colacc_{e,o} = min(colacc, scr2)
  - outputs fp16: rowacc [128, NT], colacc [2, 128, N]; all = d2 - 3072.
Host (float64): fwd2 = max(rowacc) + 3072, bwd2 = max_m(min_p min(colacc_e,
colacc_o)) + 3072, h = sqrt(max(fwd2, bwd2, 0)), mean over 24 pairs.

DMA: xt chunks on sync queue, yt chunks on scalar queue (parallel HWDGE),
small tensors/outputs on gpsimd (SWDGE); per-chunk tiles so the first
matmul only waits on the first 512 KB.
"""

import numpy as np

B, C, N, D = 8, 3, 1024, 1024
N_CORES = 8
PAIRS = B * C              # 24
PP = PAIRS // N_CORES      # 3 pairs per core
NT = N // 128              # 8 n-tiles (output partition dim)
MBS = 512                  # m block size (one PSUM bank of fp32)
MB = N // MBS              # 2 m-blocks
KC = 4                     # DoubleRow k-chunks (each 256 contraction rows)

Y2OFF = 2048.0             # host-side centering constants
X2OFF = 1024.0

_NC_CACHE = None


def _legalize_sync(nc):
    """This toolchain's walrus accepts at most ONE sync-wait per instruction;
    Tile emits several (e.g. the tail drain waits on every engine/DMA sem).
    Hoist all but the last wait of each instruction into standalone
    InstEventSemaphore instructions on the same engine, inserted just before
    it — semantically identical (the engine blocks on each in turn)."""
    import concourse.mybir as mybir

    n_split = 0
    for fn in nc.m.functions:
        for bb in fn.blocks:
            new_il = []
            for ins in bb.instructions:
                si = ins.sync_info
                if si is not None and si.on_wait and len(si.on_wait) > 1:
                    waits = list(si.on_wait)
                    for k, w in enumerate(waits[:-1]):
                        ev = mybir.InstEventSemaphore(
                            name=f"{ins.name}-evw{k}",
                            engine=ins.engine,
                            ins=[],
                            outs=[],
                            sync_info=mybir.SyncInfo(on_wait=[w], on_update=[]),
                        )
                        new_il.append(ev)
                        n_split += 1
                    si.on_wait = [waits[-1]]
                new_il.append(ins)
            bb.instructions[:] = new_il
    return n_split


def _build_nc():
    import concourse.bass as bass
    import concourse.mybir as mybir
    import concourse.tile as tile

    f16 = mybir.dt.float16
    f32 = mybir.dt.float32
    f8 = mybir.dt.float8e4
    op_add = mybir.AluOpType.add
    op_min = mybir.AluOpType.min

    nc = bass.Bass("TRN2", target_bir_lowering=True, debug=False)
    xtc_d = nc.dram_tensor("xtc", [PP, KC, 128, 2, N], f8, kind="ExternalInput").ap()
    ytc_d = nc.dram_tensor("ytc", [PP, KC, 128, 2, N], f8, kind="ExternalInput").ap()
    y2c_d = nc.dram_tensor("y2c", [PP, 1, N], f16, kind="ExternalInput").ap()
    x2_d = nc.dram_tensor("x2c", [PP, 128, NT], f32, kind="ExternalInput").ap()
    row_d = nc.dram_tensor("rowout", [PP, 128, NT], f16, kind="ExternalOutput").ap()
    col_d = nc.dram_tensor("colout", [PP, 2, 128, N], f16, kind="ExternalOutput").ap()
    # scr of the LAST n-block (d2 - 3072, fp16): row/col mins done on the
    # host so the device tail is just cast + DMA after the last matmul.
    ps7_d = nc.dram_tensor("ps7out", [PP, 128, N], f16, kind="ExternalOutput").ap()

    with tile.TileContext(nc) as tc:
        with (
            tc.tile_pool(name="const", bufs=1) as const_pool,
            tc.tile_pool(name="xy", bufs=2) as xy_pool,
            tc.tile_pool(name="small", bufs=2) as small_pool,
            tc.tile_pool(name="scr", bufs=6) as scr_pool,
            tc.tile_pool(name="ps", bufs=4, space="PSUM") as ps_pool,
        ):
            # ones on partitions 0 and 32 (concurrent fold-in row tiles)
            ones_t = const_pool.tile([33, 128], f16)
            nc.vector.memset(ones_t, 1.0)

            # HAM warm-up: keep the PE busy during the initial DMA fill so
            # the clock gate reaches 8/8 before the first real matmul.
            # Uses a regular ps-pool tile slot (released to the rotation).
            # ~20 x 256-col K=1 matmuls = ~4.3us of cold-rate PE activity,
            # enough to cross the 3.4us HAM SHORT window.
            wmov = const_pool.tile([1, 256], f16)
            nc.vector.memset(wmov, 0.0)
            wps = ps_pool.tile([128, N], f32, tag="ps")
            for w in range(20):
                nc.tensor.matmul(
                    wps[:, 0:256], ones_t[0:1, :], wmov,
                    start=True, stop=True,
                )

            for j in range(PP):
                y2c_sb = small_pool.tile([33, N], f16, tag="y2c")
                nc.gpsimd.dma_start(out=y2c_sb[0:1, :], in_=y2c_d[j])
                nc.gpsimd.dma_start(out=y2c_sb[32:33, :], in_=y2c_d[j])
                x2_sb = small_pool.tile([128, NT], f32, tag="x2")
                nc.gpsimd.dma_start(out=x2_sb, in_=x2_d[j])

                xc, yc = [], []
                for c in range(KC):
                    xt_c = xy_pool.tile([128, 2, N], f8, tag=f"xt{c}")
                    yt_c = xy_pool.tile([128, 2, N], f8, tag=f"yt{c}")
                    nc.sync.dma_start(out=xt_c, in_=xtc_d[j, c])
                    nc.scalar.dma_start(out=yt_c, in_=ytc_d[j, c])
                    xc.append(xt_c)
                    yc.append(yt_c)

                rowacc = small_pool.tile([128, NT], f16, tag="rowacc")
                colacc_e = small_pool.tile([128, N], f16, tag="colacc_e")
                colacc_o = small_pool.tile([128, N], f16, tag="colacc_o")

                for nt in range(NT):
                    nsl = slice(nt * 128, (nt + 1) * 128)
                    # [128, N] fp32 = 2 PSUM banks, address-contiguous
                    ps = ps_pool.tile([128, N], f32, tag="ps")
                    for ki in range(KC):
                        xsl = xc[ki][:, :, nsl]
                        for mb in range(MB):
                            nc.tensor.matmul(
                                ps[:, mb * MBS : (mb + 1) * MBS],
                                xsl,
                                yc[ki][:, :, mb * MBS : (mb + 1) * MBS],
                                start=(ki == 0),
                                stop=False,
                                perf_mode=mybir.MatmulPerfMode.DoubleRow,
                            )
                    # += 1 * y2c[m] (broadcast over rows): psum = y2c - 2 x.y
                    # Two K=1 fold-ins packed on array rows 0 and 32 so they
                    # run concurrently (~1x fold-in cost instead of 2x).
                    nc.tensor.matmul(
                        ps[:, 0:MBS],
                        ones_t[0:1, :],
                        y2c_sb[0:1, 0:MBS],
                        start=False,
                        stop=True,
                    )
                    nc.tensor.matmul(
                        ps[:, MBS : 2 * MBS],
                        ones_t[32:33, :],
                        y2c_sb[32:33, MBS : 2 * MBS],
                        start=False,
                        stop=True,
                    )
                    # ScalarE: scr2 = fp16(psum + x2c[nt]) = d2 - 3072
                    scr = scr_pool.tile([128, N], f16, tag="scr")
                    nc.scalar.activation(
                        out=scr,
                        in_=ps,
                        func=mybir.ActivationFunctionType.Identity,
                        bias=x2_sb[:, nt : nt + 1],
                        scale=1.0,
                    )
                    if nt == NT - 1:
                        # Last block: ship scr2; host does its row/col mins.
                        # Device tail = cast + DMA after the last matmul.
                        nc.sync.dma_start(out=ps7_d[j], in_=scr)
                        continue
                    # rowacc[:, nt] = min over m of scr2
                    nc.vector.tensor_reduce(
                        out=rowacc[:, nt : nt + 1],
                        in_=scr,
                        axis=mybir.AxisListType.X,
                        op=op_min,
                    )
                    # col path: two accumulators on VE, merged on host.
                    # colacc_e covers blocks 0,2,4,6; colacc_o covers 1,3,5.
                    if nt == 0:
                        nc.vector.tensor_copy(colacc_e, scr)
                    elif nt == 1:
                        nc.vector.tensor_copy(colacc_o, scr)
                    elif nt % 2 == 0:
                        nc.vector.tensor_tensor(colacc_e, colacc_e, scr, op_min)
                    else:
                        nc.vector.tensor_tensor(colacc_o, colacc_o, scr, op_min)
                    if nt == 5:
                        # colacc_o final (blocks 1,3,5) — overlap its DMA
                        nc.sync.dma_start(out=col_d[j, 1], in_=colacc_o)
                    elif nt == 6:
                        # colacc_e final (blocks 0,2,4,6)
                        nc.sync.dma_start(out=col_d[j, 0], in_=colacc_e)
                        nc.sync.dma_start(out=row_d[j], in_=rowacc)
    _legalize_sync(nc)
    return nc


def _prep_inputs(x, y):
    import ml_dtypes

    f8np = np.dtype(ml_dtypes.float8_e4m3)
    x32 = np.ascontiguousarray(x, dtype=np.float32).reshape(PAIRS, N, D)
    y32 = np.ascontiguousarray(y, dtype=np.float32).reshape(PAIRS, N, D)

    # fp8 chunk layout [PAIRS, KC, 128, 2, N]: element [q, c, p, o, n] =
    # op[q][k = 256c + 128o + p, n] where xt = (-2 x)^T, yt = y^T.
    xt8 = (x32.transpose(0, 2, 1) * np.float32(-2.0)).astype(f8np)  # [q, D, N]
    yt8 = y32.transpose(0, 2, 1).astype(f8np)
    xtc = np.ascontiguousarray(xt8.reshape(PAIRS, KC, 2, 128, N).transpose(0, 1, 3, 2, 4))
    ytc = np.ascontiguousarray(yt8.reshape(PAIRS, KC, 2, 128, N).transpose(0, 1, 3, 2, 4))

    x2 = np.square(x32.astype(np.float64)).sum(-1)  # [PAIRS, N]
    y2 = np.square(y32.astype(np.float64)).sum(-1)
    # x2c[q, p, t] = x2[q, t*128 + p] - X2OFF   (fp16, centered)
    x2c = np.ascontiguousarray(
        (x2 - X2OFF).reshape(PAIRS, NT, 128).transpose(0, 2, 1).astype(np.float32)
    )
    # y2c[q, 0, m] = y2[q, m] - Y2OFF  (single row, fp16)
    y2c = np.ascontiguousarray((y2 - Y2OFF).astype(np.float16)[:, None, :])
    return xtc, ytc, x2c, y2c, x2


def _run(x, y, trace=False):
    global _NC_CACHE
    from concourse.bass_utils import run_bass_kernel_spmd

    xtc, ytc, x2c, y2c, x2 = _prep_inputs(x, y)

    if _NC_CACHE is None:
        _NC_CACHE = _build_nc()
    nc = _NC_CACHE

    in_maps = []
    for i in range(N_CORES):
        q0 = i * PP
        in_maps.append(
            {
                "xtc": xtc[q0 : q0 + PP],
                "ytc": ytc[q0 : q0 + PP],
                "y2c": y2c[q0 : q0 + PP],
                "x2c": x2c[q0 : q0 + PP],
            }
        )

    res = run_bass_kernel_spmd(nc, in_maps, core_ids=list(range(N_CORES)), trace=trace)

    h2 = np.empty(PAIRS, np.float64)
    for i in range(N_CORES):
        r = res.results[i]
        for j in range(PP):
            q = i * PP + j
            # rowacc[p, t] = min_m d2 - 3072 for n = t*128 + p, t < 7
            row = r["rowout"][j].astype(np.float64)[:, : NT - 1]
            # last block: scr7 = d2 - 3072 (fp16)
            d2_7 = r["ps7out"][j].astype(np.float64) + (X2OFF + Y2OFF)
            fwd2 = max(row.max() + (X2OFF + Y2OFF), d2_7.min(axis=1).max())
            # colacc[v, p, m] = min over blocks 0..6 of d2 - 3072
            col = r["colout"][j].astype(np.float64)  # [2, 128, N]
            colmin = np.minimum(
                col.min(axis=(0, 1)) + (X2OFF + Y2OFF), d2_7.min(axis=0)
            )
            bwd2 = colmin.max()
            h2[q] = max(fwd2, bwd2, 0.0)

    ans = np.sqrt(h2).mean()
    return np.array(ans, dtype=np.float32), res


def kernel(input, target):
    out, _ = _run(np.asarray(input), np.asarray(target), trace=False)
    return out


# revision 37
# speedup vs baseline: 1.0517x; 1.0517x over previous
"""Channelwise symmetric Hausdorff distance loss on 8 Trainium2 NeuronCores.

Math (per (batch, channel) pair; x, y are [N, D] point sets):
    d2[n, m] = |x_n|^2 + |y_m|^2 - 2 x_n.y_m
    h = max( max_n min_m d(n,m), max_m min_n d(n,m) )
    answer   = mean over the B*C pairs of h.

Sharding: B*C = 24 pairs, 3 per NeuronCore (data parallel), host gathers.

v3 design (per pair, per core):
  - host-prepped: xtc/ytc fp8 in DoubleRow chunk layout [4, 128, 2, N]
    (chunk c holds contraction rows [256c, 256c+256), xt pre-scaled by -2),
    y2c fp16 [1, N] = |y_m|^2 - 2048 (centered, single row),
    x2c fp32 [128, NT] = |x_n|^2 - 1024 in per-partition layout.
  - ~28 K=1 warmup matmuls run during the initial DMA fill so the PE HAM
    clock-gate is already at 2.4 GHz when real matmuls start.
  - 8 n-blocks, each:
      PE:      8 accumulating fp8-DR matmuls (-2 x.y) + 2 K=1 fp16
               fold-ins (ones^T @ y2c) packed CONCURRENTLY on array rows
               0 and 32 -> psum = y2c - 2 x.y  (fp32)
      ScalarE: scr2 = cast(psum + x2c[b]) to fp16 (activation Identity
               with per-partition bias) = d2 - 3072
      DVE:     rowacc[:, b] = min over m of scr2   (tensor_reduce)
      DVE/GpSimd (alternating blocks): colacc_{e,o} = min(colacc, scr2)
  - outputs fp16: rowacc [128, NT], colacc [2, 128, N]; all = d2 - 3072.
Host (float64): fwd2 = max(rowacc) + 3072, bwd2 = max_m(min_p min(colacc_e,
colacc_o)) + 3072, h = sqrt(max(fwd2, bwd2, 0)), mean over 24 pairs.

DMA: xt chunks on sync queue, yt chunks on scalar queue (parallel HWDGE),
small tensors/outputs on gpsimd (SWDGE); per-chunk tiles so the first
matmul only waits on the first 512 KB.
"""

import numpy as np

B, C, N, D = 8, 3, 1024, 1024
N_CORES = 8
PAIRS = B * C              # 24
PP = PAIRS // N_CORES      # 3 pairs per core
NT = N // 128              # 8 n-tiles (output partition dim)
MBS = 512                  # m block size (one PSUM bank of fp32)
MB = N // MBS              # 2 m-blocks
KC = 4                     # DoubleRow k-chunks (each 256 contraction rows)

Y2OFF = 2048.0             # host-side centering constants
X2OFF = 1024.0

_NC_CACHE = None


def _legalize_sync(nc):
    """This toolchain's walrus accepts at most ONE sync-wait per instruction;
    Tile emits several (e.g. the tail drain waits on every engine/DMA sem).
    Hoist all but the last wait of each instruction into standalone
    InstEventSemaphore instructions on the same engine, inserted just before
    it — semantically identical (the engine blocks on each in turn)."""
    import concourse.mybir as mybir

    n_split = 0
    for fn in nc.m.functions:
        for bb in fn.blocks:
            new_il = []
            for ins in bb.instructions:
                si = ins.sync_info
                if si is not None and si.on_wait and len(si.on_wait) > 1:
                    waits = list(si.on_wait)
                    for k, w in enumerate(waits[:-1]):
                        ev = mybir.InstEventSemaphore(
                            name=f"{ins.name}-evw{k}",
                            engine=ins.engine,
                            ins=[],
                            outs=[],
                            sync_info=mybir.SyncInfo(on_wait=[w], on_update=[]),
                        )
                        new_il.append(ev)
                        n_split += 1
                    si.on_wait = [waits[-1]]
                new_il.append(ins)
            bb.instructions[:] = new_il
    return n_split


def _build_nc():
    import concourse.bass as bass
    import concourse.mybir as mybir
    import concourse.tile as tile

    f16 = mybir.dt.float16
    f32 = mybir.dt.float32
    f8 = mybir.dt.float8e4
    op_add = mybir.AluOpType.add
    op_min = mybir.AluOpType.min

    nc = bass.Bass("TRN2", target_bir_lowering=True, debug=False)
    xtc_d = nc.dram_tensor("xtc", [PP, KC, 128, 2, N], f8, kind="ExternalInput").ap()
    ytc_d = nc.dram_tensor("ytc", [PP, KC, 128, 2, N], f8, kind="ExternalInput").ap()
    y2c_d = nc.dram_tensor("y2c", [PP, 1, N], f16, kind="ExternalInput").ap()
    x2_d = nc.dram_tensor("x2c", [PP, 128, NT], f32, kind="ExternalInput").ap()
    row_d = nc.dram_tensor("rowout", [PP, 128, NT], f16, kind="ExternalOutput").ap()
    col_d = nc.dram_tensor("colout", [PP, 2, 128, N], f16, kind="ExternalOutput").ap()
    # scr of the LAST n-block (d2 - 3072, fp16): row/col mins done on the
    # host so the device tail is just cast + DMA after the last matmul.
    ps7_d = nc.dram_tensor("ps7out", [PP, 128, N], f16, kind="ExternalOutput").ap()

    with tile.TileContext(nc) as tc:
        with (
            tc.tile_pool(name="const", bufs=1) as const_pool,
            tc.tile_pool(name="xy", bufs=2) as xy_pool,
            tc.tile_pool(name="small", bufs=2) as small_pool,
            tc.tile_pool(name="scr", bufs=6) as scr_pool,
            tc.tile_pool(name="ps", bufs=4, space="PSUM") as ps_pool,
        ):
            # ones on partitions 0 and 32 (concurrent fold-in row tiles).
            # NOTE: array row-group 3 (rows 96-127) has a HW bug — a
            # tile_position=(96,0) matmul wedges the core. Max 2-3 wide.
            ones_t = const_pool.tile([33, 128], f16)
            nc.vector.memset(ones_t, 1.0)

            # HAM warm-up: keep the PE busy during the initial DMA fill so
            # the clock gate reaches 8/8 before the first real matmul.
            # Must be FULL-K matmuls — K=1 activity doesn't register as
            # PE-busy for the HAM (measured: 5us of K=1 MMs left it cold).
            wmov = const_pool.tile([128, 256], f16)
            nc.vector.memset(wmov, 1.0)
            wps = ps_pool.tile([128, N], f32, tag="ps")
            for w in range(20):
                nc.tensor.matmul(
                    wps[:, 0:256], wmov[:, 0:128], wmov,
                    start=True, stop=True,
                )

            for j in range(PP):
                y2c_sb = small_pool.tile([33, N], f16, tag="y2c")
                for bp in (0, 32):
                    nc.gpsimd.dma_start(out=y2c_sb[bp : bp + 1, :], in_=y2c_d[j])
                x2_sb = small_pool.tile([128, NT], f32, tag="x2")
                nc.gpsimd.dma_start(out=x2_sb, in_=x2_d[j])

                xc, yc = [], []
                for c in range(KC):
                    xt_c = xy_pool.tile([128, 2, N], f8, tag=f"xt{c}")
                    yt_c = xy_pool.tile([128, 2, N], f8, tag=f"yt{c}")
                    nc.sync.dma_start(out=xt_c, in_=xtc_d[j, c])
                    nc.scalar.dma_start(out=yt_c, in_=ytc_d[j, c])
                    xc.append(xt_c)
                    yc.append(yt_c)

                rowacc = small_pool.tile([128, NT], f16, tag="rowacc")
                colacc_e = small_pool.tile([128, N], f16, tag="colacc_e")
                colacc_o = small_pool.tile([128, N], f16, tag="colacc_o")

                # Process blocks in pairs: both blocks' mains, then the
                # fold-ins grouped (pairs packed on rows 0/32).
                for bp in range(NT // 2):
                    ntA, ntB = 2 * bp, 2 * bp + 1
                    psl = []
                    for nt in (ntA, ntB):
                        nsl = slice(nt * 128, (nt + 1) * 128)
                        # [128, N] fp32 = 2 PSUM banks, address-contiguous
                        ps = ps_pool.tile([128, N], f32, tag="ps")
                        psl.append(ps)
                        for ki in range(KC):
                            xsl = xc[ki][:, :, nsl]
                            for mb in range(MB):
                                nc.tensor.matmul(
                                    ps[:, mb * MBS : (mb + 1) * MBS],
                                    xsl,
                                    yc[ki][:, :, mb * MBS : (mb + 1) * MBS],
                                    start=(ki == 0),
                                    stop=False,
                                    perf_mode=mybir.MatmulPerfMode.DoubleRow,
                                )
                    # += 1 * y2c[m]: psum = y2c - 2 x.y.  Per block, two
                    # K=1 fold-ins packed concurrently on rows 0 and 32.
                    for ps, row_mb in (
                        (psl[0], ((0, 0), (32, 1))),
                        (psl[1], ((0, 0), (32, 1))),
                    ):
                        for row, mb in row_mb:
                            msl = slice(mb * MBS, (mb + 1) * MBS)
                            nc.tensor.matmul(
                                ps[:, msl],
                                ones_t[row : row + 1, :],
                                y2c_sb[row : row + 1, msl],
                                start=False,
                                stop=True,
                            )
                    for nt, ps in zip((ntA, ntB), psl):
                        # ScalarE: scr2 = fp16(psum + x2c[nt]) = d2 - 3072
                        scr = scr_pool.tile([128, N], f16, tag="scr")
                        nc.scalar.activation(
                            out=scr,
                            in_=ps,
                            func=mybir.ActivationFunctionType.Identity,
                            bias=x2_sb[:, nt : nt + 1],
                            scale=1.0,
                        )
                        if nt == NT - 1:
                            # Last block: ship scr2; host does its row/col
                            # mins. Device tail = cast + DMA after last MM.
                            nc.sync.dma_start(out=ps7_d[j], in_=scr)
                            continue
                        # rowacc[:, nt] = min over m of scr2
                        nc.vector.tensor_reduce(
                            out=rowacc[:, nt : nt + 1],
                            in_=scr,
                            axis=mybir.AxisListType.X,
                            op=op_min,
                        )
                        # col path: two accumulators on VE, host-merged.
                        # colacc_e: blocks 0,2,4,6; colacc_o: 1,3,5.
                        if nt == 0:
                            nc.vector.tensor_copy(colacc_e, scr)
                        elif nt == 1:
                            nc.vector.tensor_copy(colacc_o, scr)
                        elif nt % 2 == 0:
                            nc.vector.tensor_tensor(colacc_e, colacc_e, scr, op_min)
                        else:
                            nc.vector.tensor_tensor(colacc_o, colacc_o, scr, op_min)
                        if nt == 5:
                            # colacc_o final (blocks 1,3,5) — overlap DMA
                            nc.sync.dma_start(out=col_d[j, 1], in_=colacc_o)
                        elif nt == 6:
                            # colacc_e final (blocks 0,2,4,6)
                            nc.sync.dma_start(out=col_d[j, 0], in_=colacc_e)
                            nc.sync.dma_start(out=row_d[j], in_=rowacc)
    _legalize_sync(nc)
    return nc


def _prep_inputs(x, y):
    import ml_dtypes

    f8np = np.dtype(ml_dtypes.float8_e4m3)
    x32 = np.ascontiguousarray(x, dtype=np.float32).reshape(PAIRS, N, D)
    y32 = np.ascontiguousarray(y, dtype=np.float32).reshape(PAIRS, N, D)

    # fp8 chunk layout [PAIRS, KC, 128, 2, N]: element [q, c, p, o, n] =
    # op[q][k = 256c + 128o + p, n] where xt = (-2 x)^T, yt = y^T.
    xt8 = (x32.transpose(0, 2, 1) * np.float32(-2.0)).astype(f8np)  # [q, D, N]
    yt8 = y32.transpose(0, 2, 1).astype(f8np)
    xtc = np.ascontiguousarray(xt8.reshape(PAIRS, KC, 2, 128, N).transpose(0, 1, 3, 2, 4))
    ytc = np.ascontiguousarray(yt8.reshape(PAIRS, KC, 2, 128, N).transpose(0, 1, 3, 2, 4))

    x2 = np.square(x32.astype(np.float64)).sum(-1)  # [PAIRS, N]
    y2 = np.square(y32.astype(np.float64)).sum(-1)
    # x2c[q, p, t] = x2[q, t*128 + p] - X2OFF   (fp16, centered)
    x2c = np.ascontiguousarray(
        (x2 - X2OFF).reshape(PAIRS, NT, 128).transpose(0, 2, 1).astype(np.float32)
    )
    # y2c[q, 0, m] = y2[q, m] - Y2OFF  (single row, fp16)
    y2c = np.ascontiguousarray((y2 - Y2OFF).astype(np.float16)[:, None, :])
    return xtc, ytc, x2c, y2c, x2


def _run(x, y, trace=False):
    global _NC_CACHE
    from concourse.bass_utils import run_bass_kernel_spmd

    xtc, ytc, x2c, y2c, x2 = _prep_inputs(x, y)

    if _NC_CACHE is None:
        _NC_CACHE = _build_nc()
    nc = _NC_CACHE

    in_maps = []
    for i in range(N_CORES):
        q0 = i * PP
        in_maps.append(
            {
                "xtc": xtc[q0 : q0 + PP],
                "ytc": ytc[q0 : q0 + PP],
                "y2c": y2c[q0 : q0 + PP],
                "x2c": x2c[q0 : q0 + PP],
            }
        )

    res = run_bass_kernel_spmd(nc, in_maps, core_ids=list(range(N_CORES)), trace=trace)

    h2 = np.empty(PAIRS, np.float64)
    for i in range(N_CORES):
        r = res.results[i]
        for j in range(PP):
            q = i * PP + j
            # rowacc[p, t] = min_m d2 - 3072 for n = t*128 + p, t < 7
            row = r["rowout"][j].astype(np.float64)[:, : NT - 1]
            # last block: scr7 = d2 - 3072 (fp16)
            d2_7 = r["ps7out"][j].astype(np.float64) + (X2OFF + Y2OFF)
            fwd2 = max(row.max() + (X2OFF + Y2OFF), d2_7.min(axis=1).max())
            # colacc[v, p, m] = min over blocks 0..6 of d2 - 3072
            col = r["colout"][j].astype(np.float64)  # [2, 128, N]
            colmin = np.minimum(
                col.min(axis=(0, 1)) + (X2OFF + Y2OFF), d2_7.min(axis=0)
            )
            bwd2 = colmin.max()
            h2[q] = max(fwd2, bwd2, 0.0)

    ans = np.sqrt(h2).mean()
    return np.array(ans, dtype=np.float32), res


def kernel(input, target):
    out, _ = _run(np.asarray(input), np.asarray(target), trace=False)
    return out
